# revision 58
# baseline (speedup 1.0000x reference)
"""Trainium2 Bass kernel for nn_CausalSelfAttention_42039139893449.

Differential causal self-attention block:
  qkv = x @ ternary(W_qkv).T ; qk rmsnorm ; rope ; q*gain ; GQA expand
  y1/y2 = causal attention over head halves ; y = [y1-lam*y2, y1+lam*y2]
  out = rmsnorm(y) @ ternary(W_proj).T

Sharding over 8 NeuronCores: batch (4) x head-halves (2); per core 8 q
heads / 2 kv heads, full sequence.

Single-pass fp16 matmuls throughout (1 cycle/row in the cost model vs 3
for the old fp16 hi/lo scheme); measured end-to-end error stays well
under the 2e-2 gate.  Key structural choices:

- The differential combine y = [y1-lam*y2, y1+lam*y2] and the lambda
  factor are folded into the projection weights host-side
  (Wp'_1 = A+B, Wp'_2 = lam*(B-A)), so the device only normalizes by
  the softmax denominator and runs a plain projection.
- The output projection is row-sharded: each core projects its own 8
  heads over all 2048 output columns and a pairwise ReduceScatter(add)
  produces the final output -- no AllGather of y, no swap/combine pass.
- RMS statistics use gpsimd partition_all_reduce (Pool engine) instead
  of ones-matmuls + DRAM round trips; the rsqrt gain is folded into the
  Sqrt activation's per-partition scale/bias.
- Attention per head runs in q-blocks of 512: scores for both head
  halves land in one 2-bank PSUM tile, one Exp per (key-chunk, q-block)
  covers both halves, PV accumulates [65, 512] per half with the
  softmax denominator in row 64 (ones column in the value tile).
- Head h+1's QKV matmul groups are interleaved INTO head h's attention
  loop as elastic PE filler, and PV runs 5 key-chunks behind scores, so
  the Act-engine exp stream (the local bottleneck) never stalls the PE;
  scores/exp/PV share a 3-deep PSUM ring with the QKV tiles.
- One activation table (natural_log_exp_and_others) is pinned for the
  whole program; rsqrt is computed as exp(-0.5*ln(x)) so no 1.3us
  act-table reloads ever occur.
- exp runs with a -5 logit bias (softmax-invariant) so fp16 attention
  weights cannot overflow; projection partials, the ReduceScatter and
  the output are fp16 (host casts back to f32).

Layouts: activations transposed on device ([dim on partitions, tokens
on free]); head-dim halves packed into partitions 0-63 / 64-127.
"""
import sys

if "/opt/trn_rl_repo" not in sys.path:
    sys.path.insert(0, "/opt/trn_rl_repo")

import numpy as np

import concourse.bass as bass
import concourse.mybir as mybir
import concourse.tile as tile
from concourse import bacc
from concourse import bass_isa
from concourse import bass_utils
from concourse.hw_specs import get_activation_tables

# ---- problem constants (hardcoded) ----
B, S, DIM = 4, 1024, 2048
H, KVH, HD = 16, 4, 128
HALF = HD // 2          # 64
GS = 64
ROPE_BASE = 10000.0
QS, KVS = H * HD, KVH * HD   # 2048, 512
N_CORES = 8
HL = H // 2              # 8 q heads per core
KVL = KVH // 2           # 2 kv heads per core
REP = H // KVH           # 4
EPS = float(np.finfo(np.float32).eps)
P = 128
KC = DIM // P            # 16 contraction chunks
TT = S // P              # 8 key chunks
FTOT = HL + KVL          # 10 q+k feature tiles per core
QKCOLS = FTOT * HD       # 1280
VCOLS = KVL * HD         # 256
OCOLS = DIM // 2         # 1024 output cols per core
QB = 2                   # q blocks of 512
EXPB = -5.0              # exp logit bias (softmax-invariant f16 guard)

f32 = mybir.dt.float32
f16 = mybir.dt.float16
AF = mybir.ActivationFunctionType

_CACHE = {}
_MARKS = []


def _mark(nc, label):
    _MARKS.append((nc.next_id(), label))


# ---------------- host-side preprocessing ----------------

def _ternary_quant(w):
    wg = w.reshape(-1, GS).astype(np.float32)
    scale = np.clip(np.mean(np.abs(wg), axis=-1, keepdims=True), 1e-8, None)
    scale = scale.astype(np.float32)
    q = np.clip(np.round(wg / scale), -1.0, 1.0).astype(np.float32)
    return (q * scale).reshape(w.shape).astype(np.float32)


def _rope_tables():
    inv_freq = 1.0 / (ROPE_BASE ** (np.arange(0, HD, 2, dtype=np.float32) / HD))
    freqs = np.arange(S, dtype=np.float32)[:, None] * inv_freq[None, :]
    cos = np.cos(freqs).astype(np.float32).T   # [64, S]
    sin = np.sin(freqs).astype(np.float32).T
    cpack = np.concatenate([cos, cos], axis=0).astype(np.float16)
    spack = np.concatenate([sin, -sin], axis=0).astype(np.float16)
    return np.ascontiguousarray(cpack), np.ascontiguousarray(spack)


# ---------------- device program ----------------

def _build_program():
    dbg = bool(globals().get("DEBUG_DUMP", False))
    key = ("v3", bool(globals().get("NO_COLLECTIVE", False)), dbg)
    if key in _CACHE:
        return _CACHE[key]

    nc = bacc.Bacc("TRN2", target_bir_lowering=False, debug=False,
                   num_devices=N_CORES)

    # All activation funcs used below (Exp/Ln/Copy/Square/Identity) live in
    # the 'natural_log_exp_and_others' table.  The default per-instruction
    # table chooser picks the first matching set, which alternates between
    # exp-only and ln-only tables and inserts a 1.3us table reload around
    # every rmsnorm -- blank out the other sets (positions preserved so
    # act_func_set_id stays a valid act_info.json index) to pin the one
    # table for the whole program.
    _orig_insert = nc.insert_act_table_loads

    def _pinned_table_insert():
        import bass_rust as _bass_rust
        tables = [
            (name, s if name == "natural_log_exp_and_others" else set())
            for name, s in get_activation_tables(nc.m.arch).items()
        ]
        _bass_rust.insert_act_table_loads(nc, tables)

    nc.insert_act_table_loads = _pinned_table_insert

    def din(name, shape, dt_):
        return nc.dram_tensor(name, shape, dt_, kind="ExternalInput").ap()

    x_d = din("xT16", [DIM, S], f16)
    wqk_d = din("wqkT16", [DIM, QKCOLS], f16)
    wv_d = din("wvT16", [DIM, VCOLS], f16)
    wp_d = din("wpT16", [HL * HD, DIM], f16)
    cos_d = din("cpk16", [P, S], f16)
    sin_d = din("spk16", [P, S], f16)
    ig2_d = din("invg2", [P, FTOT], f32)
    epg_d = din("epsg2", [P, FTOT], f32)
    wsq_d = din("wsq", [P, HL], f32)
    mask_d = din("dmask16", [P, P], f16)

    out_d = nc.dram_tensor("out", [S, OCOLS], f16, kind="ExternalOutput").ap()
    if dbg:
        dbg_qk = nc.dram_tensor("dbg_qk", [P, FTOT, S], f16,
                                kind="ExternalOutput").ap()
        dbg_vp = nc.dram_tensor("dbg_vp", [P, KVL, 2, TT, HALF + 1], f16,
                                kind="ExternalOutput").ap()
        dbg_yn = nc.dram_tensor("dbg_yn", [P, HL, S], f16,
                                kind="ExternalOutput").ap()
        dbg_ss = nc.dram_tensor("dbg_ss", [1, S], f32,
                                kind="ExternalOutput").ap()
        dbg_pp = nc.dram_tensor("dbg_pp", [2, S, OCOLS], f16,
                                kind="ExternalOutput").ap()

    groups = [[2 * i, 2 * i + 1] for i in range(N_CORES // 2)]
    no_coll = bool(globals().get("NO_COLLECTIVE", False))

    with tile.TileContext(nc) as tc:
        with (
            tc.tile_pool(name="const", bufs=1) as cp,
            tc.tile_pool(name="dram", bufs=1, space="DRAM") as dp,
        ):
            dmask = cp.tile([P, P], f16)
            nc.sync.dma_start(dmask[:], mask_d[:])
            cpk = cp.tile([P, QB, 512], f16)
            nc.sync.dma_start(cpk[:], cos_d.rearrange("p (a b) -> p a b", a=QB))
            spk = cp.tile([P, QB, 512], f16)
            nc.sync.dma_start(spk[:], sin_d.rearrange("p (a b) -> p a b", a=QB))
            ig2 = cp.tile([P, FTOT], f32)
            nc.sync.dma_start(ig2[:], ig2_d[:])
            epg = cp.tile([P, FTOT], f32)
            nc.sync.dma_start(epg[:], epg_d[:])
            wsq = cp.tile([P, HL], f32)
            nc.sync.dma_start(wsq[:], wsq_d[:])
            expb = cp.tile([P, 1], f32)
            nc.vector.memset(expb[:], EXPB)
            epsc = cp.tile([P, 1], f32)
            nc.vector.memset(epsc[:], EPS)

            ssqy_in = dp.tile([1, S], f32)
            ssqy_out = dp.tile([1, S], f32)
            part_d = dp.tile([2, S, OCOLS], f16)
            red_d = dp.tile([S, OCOLS], f16)

            # diag-mask broadcast view over the two halves
            dmb = dmask[:].rearrange("p (a j) -> p a j", a=1).to_broadcast(
                [P, 2, P])

            # ---- long-lived tiles ----
            yn, free_yn = tc.tile([P, HL, QB, 512], f16, name="yn")
            qk16, free_qk16 = tc.tile([P, FTOT, QB, 512], f16, name="qk16")
            vplus, free_vplus = tc.tile([P, KVL, 2, TT, HALF + 1], f16,
                                        name="vplus")
            ssqy_acc, free_ssqy_acc = tc.tile([P, S], f32, name="ssqy_acc")
            wv_s, free_wv = tc.tile([P, KC, VCOLS], f16, name="wv")
            wp_s, free_wp = tc.tile([P, HL, DIM], f16, name="wp")
            x16, free_x16 = tc.tile([P, KC, S], f16, name="x16")

            def load_x_quarter(tq):
                nc.sync.dma_start(
                    x16[:, :, tq * 256:(tq + 1) * 256],
                    x_d[:, tq * 256:(tq + 1) * 256].rearrange(
                        "(c p) t -> p c t", p=P))

            nc.vector.memset(vplus[:, :, :, :, HALF:HALF + 1], 1.0)
            nc.vector.memset(ssqy_acc[:], 0.0)

            with tc.tile_pool(name="wk", bufs=1) as wk:
                with tc.tile_pool(name="pbig", bufs=1, space="PSUM") as pbig:

                    def load_wqkh(ft):
                        wqkh = wk.tile([P, KC, P], f16, tag="wqkh", bufs=2)
                        for halfc in range(2):
                            nc.sync.dma_start(
                                wqkh[:, halfc * 8:(halfc + 1) * 8, :],
                                wqk_d[halfc * 8 * P:(halfc + 1) * 8 * P,
                                      ft * P:(ft + 1) * P].rearrange(
                                    "(c p) f -> p c f", p=P))
                        return wqkh

                    def qkv_mm_tq(wqkh, ps, tq, splits=1):
                        w = 256 // splits
                        for sp in range(splits):
                            t0 = tq * 256 + sp * w
                            for c in range(KC):
                                nc.tensor.matmul(
                                    ps[:, tq // 2,
                                       t0 % 512:t0 % 512 + w],
                                    wqkh[:, c, :],
                                    x16[:, c, t0:t0 + w],
                                    start=(c == 0), stop=(c == KC - 1),
                                    skip_group_check=True)

                    def qkv_ft(ft, wqkh=None, first=False):
                        _mark(nc, f"qkv{ft}")
                        if wqkh is None:
                            wqkh = load_wqkh(ft)
                        ps = pbig.tile([P, QB, 512], f32, tag="big", bufs=3)
                        for tq in range(4):
                            qkv_mm_tq(wqkh, ps, tq,
                                      splits=2 if (first and tq == 0) else 1)
                        for pq in range(QB):
                            qkv_post(ft, ps, pq)

                    def qkv_fillers(ft):
                        # emission thunks for one head's QKV, interleaved into
                        # the previous head's attention as elastic PE filler
                        _mark(nc, f"qkv{ft}")
                        wqkh = load_wqkh(ft)
                        ps = pbig.tile([P, QB, 512], f32, tag="big", bufs=3)
                        return [
                            lambda: qkv_mm_tq(wqkh, ps, 0),
                            lambda: qkv_mm_tq(wqkh, ps, 1),
                            lambda: qkv_post(ft, ps, 0),
                            lambda: qkv_mm_tq(wqkh, ps, 2),
                            lambda: qkv_mm_tq(wqkh, ps, 3),
                            lambda: qkv_post(ft, ps, 1),
                        ]

                    def qkv_post(ft, ps, pq):
                        # per q-block pipeline keeps the rmsnorm+rope chain
                        # latency (which gates the next head's scores) short
                        if True:
                            qkp = qk16[:, ft, pq, :]
                            psq = ps[:, pq, :]
                            nc.vector.tensor_copy(qkp, psq)
                            # rms stats over head-dim (partitions) on Pool;
                            # square as psum * f16(psum) — one PSUM operand
                            sqf = wk.tile([P, 512], f16, tag="sqf", bufs=2)
                            nc.vector.tensor_mul(sqf[:], qkp, qkp)
                            rr = wk.tile([P, 512], f32, tag="rr", bufs=2)
                            nc.gpsimd.partition_all_reduce(
                                rr[:], sqf[:], 128, bass_isa.ReduceOp.add)
                            # rr = gain * rsqrt(mean + eps) computed as
                            # exp(-0.5*ln(.)): Ln/Exp share an act table with
                            # the attention Exp, avoiding 1.3us table reloads
                            # (Sqrt lives in a different table).  gain sign
                            # folded into W host-side, |gain| into scale/bias.
                            nc.scalar.activation(rr[:], rr[:], AF.Ln,
                                                 scale=ig2[:, ft:ft + 1],
                                                 bias=epg[:, ft:ft + 1])
                            nc.scalar.activation(rr[:], rr[:], AF.Exp,
                                                 scale=-0.5)
                            # rope (f16, 2x DVE): qk = qk*cos + swap(qk)*sin',
                            # then * rr
                            qks = wk.tile([P, 512], f16, tag="qks", bufs=2)
                            nc.sync.dma_start(qks[0:HALF], qk16[HALF:P, ft, pq])
                            nc.sync.dma_start(qks[HALF:P], qk16[0:HALF, ft, pq])
                            nc.vector.tensor_mul(qks[:], qks[:], spk[:, pq])
                            nc.vector.tensor_mul(qkp, qkp, cpk[:, pq])
                            nc.vector.tensor_add(qkp, qkp, qks[:])
                            nc.vector.tensor_mul(qkp, qkp, rr[:])

                    # startup: interleave weight/x loads so PE starts early
                    wqkh8 = wk.tile([P, KC, P], f16, tag="wqkh", bufs=2)
                    nc.sync.dma_start(
                        wqkh8[:, 0:8, :],
                        wqk_d[0:8 * P, HL * P:(HL + 1) * P].rearrange(
                            "(c p) f -> p c f", p=P))
                    load_x_quarter(0)
                    nc.sync.dma_start(
                        wqkh8[:, 8:16, :],
                        wqk_d[8 * P:16 * P, HL * P:(HL + 1) * P].rearrange(
                            "(c p) f -> p c f", p=P))
                    load_x_quarter(1)
                    wqkh9 = load_wqkh(HL + 1)
                    load_x_quarter(2)
                    load_x_quarter(3)
                    qkv_ft(HL, wqkh8)
                    nc.sync.dma_start(wv_s[:],
                                      wv_d.rearrange("(c p) f -> p c f", p=P))
                    qkv_ft(HL + 1, wqkh9)

                    # V projection: [tokens, vfeats] per key chunk,
                    # with head 0's QKV interleaved as filler
                    with tc.tile_pool(name="pv", bufs=1, space="PSUM") as pv:
                        _mark(nc, "vproj")
                        fill0 = qkv_fillers(0)
                        for kc in range(TT):
                            psv = pv.tile([P, VCOLS], f32, tag="psv", bufs=2)
                            for c in range(KC):
                                nc.tensor.matmul(
                                    psv[:], x16[:, c, kc * P:(kc + 1) * P],
                                    wv_s[:, c, :],
                                    start=(c == 0), stop=(c == KC - 1),
                                    skip_group_check=True)
                            nc.scalar.activation(
                                vplus[:, :, :, kc, 0:HALF],
                                psv[:].rearrange("p (kv hf f) -> p kv hf f",
                                                 kv=KVL, hf=2), AF.Copy)
                            if fill0 and kc % 2 == 1:
                                fill0.pop(0)()
                        for f in fill0:
                            f()

                    with tc.tile_pool(name="pat", bufs=1, space="PSUM") as pat:

                        def attention(h, fill=()):
                            fill = list(fill)
                            _mark(nc, f"attn{h}")
                            kv = h // REP
                            kft = HL + kv
                            for qb in range(QB):
                                kcs = range(min(TT, (qb + 1) * 4))
                                yps = [pat.tile([HALF + 1, 512], f32,
                                                tag=f"yps{s}", bufs=1,
                                                name=f"yps{s}")
                                       for s in range(2)]

                                def pv_mm(kc, pt):
                                    c0 = max(0, kc * P - qb * 512)
                                    for s in range(2):
                                        nc.tensor.matmul(
                                            yps[s][:, c0:512],
                                            vplus[:, kv, s, kc, :],
                                            pt[:, s, c0:512],
                                            start=(kc == kcs[0]),
                                            stop=(kc == kcs[-1]),
                                            skip_group_check=True)

                                # scores run 2 key-chunks ahead of PV so the
                                # Act-engine exp latency stays off the PE path
                                pend = []
                                for kc in kcs:
                                    k0 = kc * P
                                    c0 = max(0, k0 - qb * 512)
                                    st = pbig.tile([P, QB, 512], f32,
                                                   tag="big", bufs=3)
                                    pt = wk.tile([P, 2, 512], f16, tag="pt",
                                                 bufs=6)
                                    for s in range(2):
                                        pb = s * HALF
                                        nc.tensor.matmul(
                                            st[:, s, c0:512],
                                            qk16[pb:pb + HALF, kft, kc // 4,
                                                 (k0 % 512):(k0 % 512) + P],
                                            qk16[pb:pb + HALF, h, qb, c0:512],
                                            start=True, stop=True,
                                            skip_group_check=True)
                                    nc.scalar.activation(
                                        pt[:, :, c0:512], st[:, :, c0:512],
                                        AF.Exp, scale=float(1.0 / np.sqrt(HALF)),
                                        bias=expb[:, 0:1])
                                    if qb * 512 <= k0:
                                        nc.vector.tensor_mul(
                                            pt[:, :, c0:c0 + P],
                                            pt[:, :, c0:c0 + P], dmb)
                                    pend.append((kc, pt))
                                    if len(pend) > 5:
                                        pv_mm(*pend.pop(0))
                                    if fill and kc % 2 == 1:
                                        fill.pop(0)()
                                for it in pend:
                                    pv_mm(*it)
                                _mark(nc, f"norm{h}.{qb}")
                                # normalize by softmax denominator (row
                                # 64); s=1 first: its shift-DMA is the tail
                                for s in (1, 0):
                                    dr = wk.tile([1, 512], f32, tag="dr",
                                                 bufs=4)
                                    nc.vector.reciprocal(
                                        dr[:], yps[s][HALF:HALF + 1, :])
                                    rb = wk.tile([HALF, 512], f32, tag="rb",
                                                 bufs=4)
                                    nc.gpsimd.partition_broadcast(
                                        rb[:], dr[:], channels=HALF)
                                    if s == 0:
                                        nc.vector.tensor_mul(
                                            yn[0:HALF, h, qb, :],
                                            yps[s][0:HALF, :], rb[:])
                                    else:
                                        ystg = wk.tile([HALF, 512], f16,
                                                       tag="ystg", bufs=2)
                                        nc.vector.tensor_mul(
                                            ystg[:], yps[s][0:HALF, :], rb[:])
                                        nc.sync.dma_start(
                                            yn[HALF:P, h, qb, :], ystg[:])
                            # final-rms stats for this head (lambda-weighted),
                            # on DVE: sqy = (yn * w) * yn, per q-block so the
                            # last chain after head 7 is short
                            for sq_qb in range(QB):
                                sqy = wk.tile([P, 512], f32, tag="sqy", bufs=2)
                                ynh = yn[:, h, sq_qb, :]
                                nc.vector.scalar_tensor_tensor(
                                    sqy[:], ynh, wsq[:, h:h + 1], ynh,
                                    mybir.AluOpType.mult, mybir.AluOpType.mult)
                                nc.gpsimd.tensor_add(
                                    ssqy_acc[:, sq_qb * 512:(sq_qb + 1) * 512],
                                    ssqy_acc[:, sq_qb * 512:(sq_qb + 1) * 512],
                                    sqy[:])
                            # stream one projection-weight chunk per head
                            nc.sync.dma_start(wp_s[:, h],
                                              wp_d[h * P:(h + 1) * P, :])

                        for h in range(HL):
                            fill = qkv_fillers(h + 1) if h < HL - 1 else ()
                            attention(h, fill)

                    # ---- final rms + pairwise collectives + projection ----
                    _mark(nc, "rmsy")
                    ssqb = wk.tile([P, S], f32, tag="ssqbc", bufs=1)
                    nc.gpsimd.partition_all_reduce(ssqb[:], ssqy_acc[:], 128,
                                                   bass_isa.ReduceOp.add)
                    nc.sync.dma_start(ssqy_in[:], ssqb[0:1, :])
                    if no_coll:
                        nc.sync.dma_start(ssqy_out[:], ssqb[0:1, :])
                    else:
                        nc.gpsimd.collective_compute(
                            "AllReduce", mybir.AluOpType.add,
                            ins=[ssqy_in.opt()], outs=[ssqy_out.opt()],
                            replica_groups=groups)
                    rry = wk.tile([P, TT], f32, tag="rry", bufs=1)
                    nc.sync.dma_start(
                        rry[:],
                        ssqy_out[0:1, :].rearrange("a (t p) -> (a p) t", p=P))
                    nc.scalar.activation(rry[:], rry[:], AF.Ln,
                                         scale=1.0 / DIM, bias=epsc[:, 0:1])
                    nc.scalar.activation(rry[:], rry[:], AF.Exp, scale=-0.5)

                    # projection reuses the big PSUM ring (no pool barrier
                    # between attention tail and first proj matmul)
                    _mark(nc, "proj")
                    for oc in range(4):
                        for t_ in range(TT):
                            psb = pbig.tile([P, QB, 512], f32, tag="big",
                                            bufs=3)
                            pso = psb[:, 0, :]
                            for c in range(HL):
                                nc.tensor.matmul(
                                    pso,
                                    yn[:, c, t_ // 4,
                                       (t_ % 4) * P:(t_ % 4 + 1) * P],
                                    wp_s[:, c, oc * 512:(oc + 1) * 512],
                                    start=(c == 0), stop=(c == HL - 1),
                                    skip_group_check=True)
                            # unscaled PSUM->SBUF copy first (DVE) so the
                            # PSUM ring never waits on the rry chain
                            osr = wk.tile([P, 512], f32, tag="osr", bufs=9)
                            nc.vector.tensor_copy(osr[:], pso)
                            osb = wk.tile([P, 512], f16, tag="osb", bufs=4)
                            nc.scalar.activation(osb[:], osr[:], AF.Copy,
                                                 scale=rry[:, t_:t_ + 1])
                            nc.sync.dma_start(
                                part_d[oc // 2, t_ * P:(t_ + 1) * P,
                                       (oc % 2) * 512:(oc % 2) * 512 + 512],
                                osb[:])
                            if no_coll and oc < 2:
                                # stub for the pairwise ReduceScatter:
                                # stream own-half partials to the output
                                nc.sync.dma_start(
                                    out_d[t_ * P:(t_ + 1) * P,
                                          oc * 512:(oc + 1) * 512],
                                    osb[:])
                    if not no_coll:
                        nc.gpsimd.collective_compute(
                            "ReduceScatter", mybir.AluOpType.add,
                            ins=[part_d.opt()], outs=[red_d.opt()],
                            replica_groups=groups)
                        nc.sync.dma_start(out_d[:, :], red_d[:])

            if dbg:
                nc.sync.dma_start(
                    dbg_qk.rearrange("p f (a b) -> p f a b", a=QB), qk16[:])
                nc.sync.dma_start(dbg_vp[:, :, :, :, :], vplus[:])
                nc.sync.dma_start(
                    dbg_yn.rearrange("p h (a b) -> p h a b", a=QB), yn[:])
                nc.sync.dma_start(dbg_ss[:], ssqy_in[:])
                nc.sync.dma_start(dbg_pp[:, :, :], part_d[:])

            free_x16()
            free_wp()
            free_wv()
            free_ssqy_acc()
            free_vplus()
            free_qk16()
            free_yn()

    nc.compile()
    _CACHE[key] = nc
    return nc


# ---------------- host wrapper ----------------

def _prep_inputs(x, w_qkv, w_proj, q_gain, diff_lambda):
    x = np.asarray(x, dtype=np.float32)
    wq = _ternary_quant(np.asarray(w_qkv, dtype=np.float32))
    wp = _ternary_quant(np.asarray(w_proj, dtype=np.float32))
    gain = np.asarray(q_gain, dtype=np.float32)
    lam = np.asarray(diff_lambda, dtype=np.float32)
    cpk, spk = _rope_tables()

    # causal mask for diagonal 128x128 blocks in scores^T layout:
    # element (key p, query j) valid iff j >= p
    dmask = (np.arange(P)[None, :] >= np.arange(P)[:, None]).astype(np.float16)
    dmask = np.ascontiguousarray(dmask)

    in_maps = []
    for core in range(N_CORES):
        b, hh = core // 2, core % 2
        q_rows = wq[hh * HL * HD:(hh + 1) * HL * HD].copy()    # [1024, 2048]
        k_rows = wq[QS + hh * KVL * HD: QS + (hh + 1) * KVL * HD]
        v_rows = wq[QS + KVS + hh * KVL * HD: QS + KVS + (hh + 1) * KVL * HD]
        gains = gain[hh * HL:(hh + 1) * HL]
        lams = lam[hh * HL:(hh + 1) * HL]

        # fold sign(gain) into the q weight rows, |gain| into the rms scale
        sg = np.sign(gains).astype(np.float32)
        sg[sg == 0] = 1.0
        q_rows *= np.repeat(sg, HD)[:, None]
        ag = np.maximum(np.abs(gains), 1e-30).astype(np.float32)

        wqk_T = np.ascontiguousarray(
            np.concatenate([q_rows, k_rows], axis=0).T.astype(np.float16))
        wv_T = np.ascontiguousarray(v_rows.T.astype(np.float16))
        x16 = np.ascontiguousarray(x[b].T.astype(np.float16))

        # projection weights with the differential combine + lambda folded:
        # y-dim order matches yn: per local head, [half1(64) | half2(64)]
        wpt = np.empty((HL * HD, DIM), np.float32)
        for i in range(HL):
            hg = hh * HL + i
            A = wp[:, hg * HD: hg * HD + HALF]           # [2048, 64]
            Bc = wp[:, hg * HD + HALF: (hg + 1) * HD]
            wpt[i * HD: i * HD + HALF] = (A + Bc).T
            wpt[i * HD + HALF: (i + 1) * HD] = (lams[i] * (Bc - A)).T
        wpt16 = np.ascontiguousarray(wpt.astype(np.float16))

        invg2 = np.full((P, FTOT), 1.0 / HD, np.float32)
        invg2[:, :HL] = (1.0 / (HD * ag * ag))[None, :]
        epsg = np.full((P, FTOT), EPS, np.float32)
        epsg[:, :HL] = (EPS / (ag * ag))[None, :]
        # stats weights (applied as (yn*w)*yn): 2 for half1, 2*lam^2 half2
        wsq = np.empty((P, HL), np.float32)
        wsq[0:HALF, :] = 2.0
        wsq[HALF:P, :] = (2.0 * lams * lams)[None, :]

        m = {
            "xT16": x16, "wqkT16": wqk_T, "wvT16": wv_T, "wpT16": wpt16,
            "cpk16": cpk, "spk16": spk,
            "invg2": np.ascontiguousarray(invg2),
            "epsg2": np.ascontiguousarray(epsg),
            "wsq": np.ascontiguousarray(wsq),
            "dmask16": dmask,
        }
        in_maps.append(m)
    return in_maps


def kernel(x, w_qkv, w_proj, q_gain, diff_lambda):
    nc = _build_program()
    in_maps = _prep_inputs(x, w_qkv, w_proj, q_gain, diff_lambda)
    last_err = None
    for attempt in range(3):
        try:
            res = bass_utils.run_bass_kernel_spmd(
                nc, in_maps, core_ids=list(range(N_CORES)))
            break
        except Exception as e:  # transient device wedges recover on retry
            last_err = e
            import time as _time
            _time.sleep(2.0)
    else:
        raise last_err
    out = np.empty((B, S, DIM), dtype=np.float32)
    for core in range(N_CORES):
        b, hh = core // 2, core % 2
        out[b, :, hh * OCOLS:(hh + 1) * OCOLS] = (
            res.results[core]["out"].astype(np.float32))
    return out


# revision 59
# speedup vs baseline: 1.0081x; 1.0081x over previous
"""Trainium2 Bass kernel for nn_CausalSelfAttention_42039139893449.

Differential causal self-attention block:
  qkv = x @ ternary(W_qkv).T ; qk rmsnorm ; rope ; q*gain ; GQA expand
  y1/y2 = causal attention over head halves ; y = [y1-lam*y2, y1+lam*y2]
  out = rmsnorm(y) @ ternary(W_proj).T

Sharding over 8 NeuronCores: batch (4) x head-halves (2); per core 8 q
heads / 2 kv heads, full sequence.

Single-pass fp16 matmuls throughout (1 cycle/row in the cost model vs 3
for the old fp16 hi/lo scheme); measured end-to-end error stays well
under the 2e-2 gate.  Key structural choices:

- The differential combine y = [y1-lam*y2, y1+lam*y2] and the lambda
  factor are folded into the projection weights host-side
  (Wp'_1 = A+B, Wp'_2 = lam*(B-A)), so the device only normalizes by
  the softmax denominator and runs a plain projection.
- The output projection is row-sharded: each core projects its own 8
  heads over all 2048 output columns and a pairwise ReduceScatter(add)
  produces the final output -- no AllGather of y, no swap/combine pass.
- RMS statistics use gpsimd partition_all_reduce (Pool engine) instead
  of ones-matmuls + DRAM round trips; the rsqrt gain is folded into the
  Sqrt activation's per-partition scale/bias.
- Attention per head runs in q-blocks of 512: scores for both head
  halves land in one 2-bank PSUM tile, one Exp per (key-chunk, q-block)
  covers both halves, PV accumulates [65, 512] per half with the
  softmax denominator in row 64 (ones column in the value tile).
- Head h+1's QKV matmul groups are interleaved INTO head h's attention
  loop as elastic PE filler, and PV runs 5 key-chunks behind scores, so
  the Act-engine exp stream (the local bottleneck) never stalls the PE;
  scores/exp/PV share a 3-deep PSUM ring with the QKV tiles.
- One activation table (natural_log_exp_and_others) is pinned for the
  whole program; rsqrt is computed as exp(-0.5*ln(x)) so no 1.3us
  act-table reloads ever occur.
- exp runs with a -5 logit bias (softmax-invariant) so fp16 attention
  weights cannot overflow; projection partials, the ReduceScatter and
  the output are fp16 (host casts back to f32).

Layouts: activations transposed on device ([dim on partitions, tokens
on free]); head-dim halves packed into partitions 0-63 / 64-127.
"""
import sys

if "/opt/trn_rl_repo" not in sys.path:
    sys.path.insert(0, "/opt/trn_rl_repo")

import numpy as np

import concourse.bass as bass
import concourse.mybir as mybir
import concourse.tile as tile
from concourse import bacc
from concourse import bass_isa
from concourse import bass_utils
from concourse.hw_specs import get_activation_tables

# ---- problem constants (hardcoded) ----
B, S, DIM = 4, 1024, 2048
H, KVH, HD = 16, 4, 128
HALF = HD // 2          # 64
GS = 64
ROPE_BASE = 10000.0
QS, KVS = H * HD, KVH * HD   # 2048, 512
N_CORES = 8
HL = H // 2              # 8 q heads per core
KVL = KVH // 2           # 2 kv heads per core
REP = H // KVH           # 4
EPS = float(np.finfo(np.float32).eps)
P = 128
KC = DIM // P            # 16 contraction chunks
TT = S // P              # 8 key chunks
FTOT = HL + KVL          # 10 q+k feature tiles per core
QKCOLS = FTOT * HD       # 1280
VCOLS = KVL * HD         # 256
OCOLS = DIM // 2         # 1024 output cols per core
QB = 2                   # q blocks of 512
EXPB = -5.0              # exp logit bias (softmax-invariant f16 guard)

f32 = mybir.dt.float32
f16 = mybir.dt.float16
AF = mybir.ActivationFunctionType

_CACHE = {}
_MARKS = []


def _mark(nc, label):
    _MARKS.append((nc.next_id(), label))


# ---------------- host-side preprocessing ----------------

def _ternary_quant(w):
    wg = w.reshape(-1, GS).astype(np.float32)
    scale = np.clip(np.mean(np.abs(wg), axis=-1, keepdims=True), 1e-8, None)
    scale = scale.astype(np.float32)
    q = np.clip(np.round(wg / scale), -1.0, 1.0).astype(np.float32)
    return (q * scale).reshape(w.shape).astype(np.float32)


def _rope_tables():
    inv_freq = 1.0 / (ROPE_BASE ** (np.arange(0, HD, 2, dtype=np.float32) / HD))
    freqs = np.arange(S, dtype=np.float32)[:, None] * inv_freq[None, :]
    cos = np.cos(freqs).astype(np.float32).T   # [64, S]
    sin = np.sin(freqs).astype(np.float32).T
    cpack = np.concatenate([cos, cos], axis=0).astype(np.float16)
    spack = np.concatenate([sin, -sin], axis=0).astype(np.float16)
    return np.ascontiguousarray(cpack), np.ascontiguousarray(spack)


# ---------------- device program ----------------

def _build_program():
    dbg = bool(globals().get("DEBUG_DUMP", False))
    key = ("v3", bool(globals().get("NO_COLLECTIVE", False)), dbg)
    if key in _CACHE:
        return _CACHE[key]

    nc = bacc.Bacc("TRN2", target_bir_lowering=False, debug=False,
                   num_devices=N_CORES)

    # All activation funcs used below (Exp/Ln/Copy/Square/Identity) live in
    # the 'natural_log_exp_and_others' table.  The default per-instruction
    # table chooser picks the first matching set, which alternates between
    # exp-only and ln-only tables and inserts a 1.3us table reload around
    # every rmsnorm -- blank out the other sets (positions preserved so
    # act_func_set_id stays a valid act_info.json index) to pin the one
    # table for the whole program.
    _orig_insert = nc.insert_act_table_loads

    def _pinned_table_insert():
        import bass_rust as _bass_rust
        tables = [
            (name, s if name == "natural_log_exp_and_others" else set())
            for name, s in get_activation_tables(nc.m.arch).items()
        ]
        _bass_rust.insert_act_table_loads(nc, tables)

    nc.insert_act_table_loads = _pinned_table_insert

    def din(name, shape, dt_):
        return nc.dram_tensor(name, shape, dt_, kind="ExternalInput").ap()

    x_d = din("xT16", [DIM, S], f16)
    wqk_d = din("wqkT16", [DIM, QKCOLS], f16)
    wv_d = din("wvT16", [DIM, VCOLS], f16)
    wp_d = din("wpT16", [HL * HD, DIM], f16)
    cos_d = din("cpk16", [P, S], f16)
    sin_d = din("spk16", [P, S], f16)
    ig2_d = din("invg2", [P, FTOT], f32)
    epg_d = din("epsg2", [P, FTOT], f32)
    wsq_d = din("wsq", [P, HL], f32)
    mask_d = din("dmask16", [P, P], f16)

    out_d = nc.dram_tensor("out", [S, OCOLS], f16, kind="ExternalOutput").ap()
    if dbg:
        dbg_qk = nc.dram_tensor("dbg_qk", [P, FTOT, S], f16,
                                kind="ExternalOutput").ap()
        dbg_vp = nc.dram_tensor("dbg_vp", [P, KVL, 2, TT, HALF + 1], f16,
                                kind="ExternalOutput").ap()
        dbg_yn = nc.dram_tensor("dbg_yn", [P, HL, S], f16,
                                kind="ExternalOutput").ap()
        dbg_ss = nc.dram_tensor("dbg_ss", [1, S], f32,
                                kind="ExternalOutput").ap()
        dbg_pp = nc.dram_tensor("dbg_pp", [2, S, OCOLS], f16,
                                kind="ExternalOutput").ap()

    groups = [[2 * i, 2 * i + 1] for i in range(N_CORES // 2)]
    no_coll = bool(globals().get("NO_COLLECTIVE", False))

    with tile.TileContext(nc) as tc:
        with (
            tc.tile_pool(name="const", bufs=1) as cp,
            tc.tile_pool(name="dram", bufs=1, space="DRAM") as dp,
        ):
            dmask = cp.tile([P, P], f16)
            nc.sync.dma_start(dmask[:], mask_d[:])
            cpk = cp.tile([P, QB, 512], f16)
            nc.sync.dma_start(cpk[:], cos_d.rearrange("p (a b) -> p a b", a=QB))
            spk = cp.tile([P, QB, 512], f16)
            nc.sync.dma_start(spk[:], sin_d.rearrange("p (a b) -> p a b", a=QB))
            ig2 = cp.tile([P, FTOT], f32)
            nc.sync.dma_start(ig2[:], ig2_d[:])
            epg = cp.tile([P, FTOT], f32)
            nc.sync.dma_start(epg[:], epg_d[:])
            wsq = cp.tile([P, HL], f32)
            nc.sync.dma_start(wsq[:], wsq_d[:])
            expb = cp.tile([P, 1], f32)
            nc.vector.memset(expb[:], EXPB)
            epsc = cp.tile([P, 1], f32)
            nc.vector.memset(epsc[:], EPS)

            ssqy_in = dp.tile([1, S], f32)
            ssqy_out = dp.tile([1, S], f32)
            part_d = dp.tile([2, S, OCOLS], f16)
            red_d = dp.tile([S, OCOLS], f16)

            # diag-mask broadcast view over the two halves
            dmb = dmask[:].rearrange("p (a j) -> p a j", a=1).to_broadcast(
                [P, 2, P])

            # ---- long-lived tiles ----
            yn, free_yn = tc.tile([P, HL, QB, 512], f16, name="yn")
            qk16, free_qk16 = tc.tile([P, FTOT, QB, 512], f16, name="qk16")
            vplus, free_vplus = tc.tile([P, KVL, 2, TT, HALF + 1], f16,
                                        name="vplus")
            ssqy_acc, free_ssqy_acc = tc.tile([P, S], f32, name="ssqy_acc")
            wv_s, free_wv = tc.tile([P, KC, VCOLS], f16, name="wv")
            wp_s, free_wp = tc.tile([P, HL, DIM], f16, name="wp")
            x16, free_x16 = tc.tile([P, KC, S], f16, name="x16")

            def load_x_quarter(tq):
                nc.sync.dma_start(
                    x16[:, :, tq * 256:(tq + 1) * 256],
                    x_d[:, tq * 256:(tq + 1) * 256].rearrange(
                        "(c p) t -> p c t", p=P))

            nc.vector.memset(vplus[:, :, :, :, HALF:HALF + 1], 1.0)
            nc.vector.memset(ssqy_acc[:], 0.0)

            with tc.tile_pool(name="wk", bufs=1) as wk:
                with tc.tile_pool(name="pbig", bufs=1, space="PSUM") as pbig:

                    def load_wqkh(ft):
                        wqkh = wk.tile([P, KC, P], f16, tag="wqkh", bufs=2)
                        for halfc in range(2):
                            nc.sync.dma_start(
                                wqkh[:, halfc * 8:(halfc + 1) * 8, :],
                                wqk_d[halfc * 8 * P:(halfc + 1) * 8 * P,
                                      ft * P:(ft + 1) * P].rearrange(
                                    "(c p) f -> p c f", p=P))
                        return wqkh

                    def qkv_mm_tq(wqkh, ps, tq, splits=1):
                        w = 256 // splits
                        for sp in range(splits):
                            t0 = tq * 256 + sp * w
                            for c in range(KC):
                                nc.tensor.matmul(
                                    ps[:, tq // 2,
                                       t0 % 512:t0 % 512 + w],
                                    wqkh[:, c, :],
                                    x16[:, c, t0:t0 + w],
                                    start=(c == 0), stop=(c == KC - 1),
                                    skip_group_check=True)

                    def qkv_ft(ft, wqkh=None, first=False):
                        _mark(nc, f"qkv{ft}")
                        if wqkh is None:
                            wqkh = load_wqkh(ft)
                        ps = pbig.tile([P, QB, 512], f32, tag="big", bufs=3)
                        for tq in range(4):
                            qkv_mm_tq(wqkh, ps, tq,
                                      splits=2 if (first and tq == 0) else 1)
                        for pq in range(QB):
                            qkv_post(ft, ps, pq)

                    def qkv_fillers(ft):
                        # emission thunks for one head's QKV, interleaved into
                        # the previous head's attention as elastic PE filler
                        _mark(nc, f"qkv{ft}")
                        wqkh = load_wqkh(ft)
                        ps = pbig.tile([P, QB, 512], f32, tag="big", bufs=3)
                        return [
                            lambda: qkv_mm_tq(wqkh, ps, 0),
                            lambda: qkv_mm_tq(wqkh, ps, 1),
                            lambda: qkv_post(ft, ps, 0),
                            lambda: qkv_mm_tq(wqkh, ps, 2),
                            lambda: qkv_mm_tq(wqkh, ps, 3),
                            lambda: qkv_post(ft, ps, 1),
                        ]

                    def qkv_post(ft, ps, pq):
                        # per q-block pipeline keeps the rmsnorm+rope chain
                        # latency (which gates the next head's scores) short
                        if True:
                            qkp = qk16[:, ft, pq, :]
                            psq = ps[:, pq, :]
                            nc.vector.tensor_copy(qkp, psq)
                            # rms stats over head-dim (partitions) on Pool;
                            # square as psum * f16(psum) — one PSUM operand
                            sqf = wk.tile([P, 512], f16, tag="sqf", bufs=2)
                            nc.vector.tensor_mul(sqf[:], qkp, qkp)
                            rr = wk.tile([P, 512], f32, tag="rr", bufs=2)
                            nc.gpsimd.partition_all_reduce(
                                rr[:], sqf[:], 128, bass_isa.ReduceOp.add)
                            # rr = gain * rsqrt(mean + eps) computed as
                            # exp(-0.5*ln(.)): Ln/Exp share an act table with
                            # the attention Exp, avoiding 1.3us table reloads
                            # (Sqrt lives in a different table).  gain sign
                            # folded into W host-side, |gain| into scale/bias.
                            nc.scalar.activation(rr[:], rr[:], AF.Ln,
                                                 scale=ig2[:, ft:ft + 1],
                                                 bias=epg[:, ft:ft + 1])
                            nc.scalar.activation(rr[:], rr[:], AF.Exp,
                                                 scale=-0.5)
                            # rope (f16, 2x DVE): qk = qk*cos + swap(qk)*sin',
                            # then * rr
                            qks = wk.tile([P, 512], f16, tag="qks", bufs=2)
                            nc.sync.dma_start(qks[0:HALF], qk16[HALF:P, ft, pq])
                            nc.sync.dma_start(qks[HALF:P], qk16[0:HALF, ft, pq])
                            nc.vector.tensor_mul(qks[:], qks[:], spk[:, pq])
                            nc.vector.tensor_mul(qkp, qkp, cpk[:, pq])
                            nc.vector.tensor_add(qkp, qkp, qks[:])
                            nc.vector.tensor_mul(qkp, qkp, rr[:])

                    # startup: interleave weight/x loads so PE starts early
                    wqkh8 = wk.tile([P, KC, P], f16, tag="wqkh", bufs=2)
                    nc.sync.dma_start(
                        wqkh8[:, 0:8, :],
                        wqk_d[0:8 * P, HL * P:(HL + 1) * P].rearrange(
                            "(c p) f -> p c f", p=P))
                    load_x_quarter(0)
                    nc.sync.dma_start(
                        wqkh8[:, 8:16, :],
                        wqk_d[8 * P:16 * P, HL * P:(HL + 1) * P].rearrange(
                            "(c p) f -> p c f", p=P))
                    nc.sync.dma_start(wv_s[:],
                                      wv_d.rearrange("(c p) f -> p c f", p=P))
                    load_x_quarter(1)
                    load_x_quarter(2)
                    wqkh9 = load_wqkh(HL + 1)
                    load_x_quarter(3)

                    # V chunks interleave with the k-head QKV groups: each
                    # needs only wv + the x quarters already landed, so PE
                    # has work during every input-DMA wait; the last two V
                    # chunks cover head 0's rope chain.
                    with tc.tile_pool(name="pv", bufs=1, space="PSUM") as pv:
                        _mark(nc, "vproj")

                        def v_chunk(kc):
                            psv = pv.tile([P, VCOLS], f32, tag="psv", bufs=2)
                            for c in range(KC):
                                nc.tensor.matmul(
                                    psv[:], x16[:, c, kc * P:(kc + 1) * P],
                                    wv_s[:, c, :],
                                    start=(c == 0), stop=(c == KC - 1),
                                    skip_group_check=True)
                            nc.scalar.activation(
                                vplus[:, :, :, kc, 0:HALF],
                                psv[:].rearrange("p (kv hf f) -> p kv hf f",
                                                 kv=KVL, hf=2), AF.Copy)

                        _mark(nc, f"qkv{HL}")
                        ps8 = pbig.tile([P, QB, 512], f32, tag="big", bufs=3)
                        qkv_mm_tq(wqkh8, ps8, 0)
                        v_chunk(0)
                        v_chunk(1)
                        qkv_mm_tq(wqkh8, ps8, 1)
                        v_chunk(2)
                        v_chunk(3)
                        qkv_mm_tq(wqkh8, ps8, 2)
                        v_chunk(4)
                        v_chunk(5)
                        qkv_mm_tq(wqkh8, ps8, 3)
                        qkv_post(HL, ps8, 0)
                        qkv_post(HL, ps8, 1)
                        qkv_ft(HL + 1, wqkh9)
                        _mark(nc, "qkv0")
                        wqkh0 = load_wqkh(0)
                        ps0 = pbig.tile([P, QB, 512], f32, tag="big", bufs=3)
                        for tq in range(4):
                            qkv_mm_tq(wqkh0, ps0, tq)
                        qkv_post(0, ps0, 0)
                        qkv_post(0, ps0, 1)
                        v_chunk(6)
                        v_chunk(7)

                    with tc.tile_pool(name="pat", bufs=1, space="PSUM") as pat:

                        def attention(h, fill=()):
                            fill = list(fill)
                            _mark(nc, f"attn{h}")
                            kv = h // REP
                            kft = HL + kv
                            for qb in range(QB):
                                kcs = range(min(TT, (qb + 1) * 4))
                                yps = [pat.tile([HALF + 1, 512], f32,
                                                tag=f"yps{s}", bufs=1,
                                                name=f"yps{s}")
                                       for s in range(2)]

                                def pv_mm(kc, pt):
                                    c0 = max(0, kc * P - qb * 512)
                                    for s in range(2):
                                        nc.tensor.matmul(
                                            yps[s][:, c0:512],
                                            vplus[:, kv, s, kc, :],
                                            pt[:, s, c0:512],
                                            start=(kc == kcs[0]),
                                            stop=(kc == kcs[-1]),
                                            skip_group_check=True)

                                # scores run 2 key-chunks ahead of PV so the
                                # Act-engine exp latency stays off the PE path
                                pend = []
                                for kc in kcs:
                                    k0 = kc * P
                                    c0 = max(0, k0 - qb * 512)
                                    st = pbig.tile([P, QB, 512], f32,
                                                   tag="big", bufs=3)
                                    pt = wk.tile([P, 2, 512], f16, tag="pt",
                                                 bufs=6)
                                    for s in range(2):
                                        pb = s * HALF
                                        nc.tensor.matmul(
                                            st[:, s, c0:512],
                                            qk16[pb:pb + HALF, kft, kc // 4,
                                                 (k0 % 512):(k0 % 512) + P],
                                            qk16[pb:pb + HALF, h, qb, c0:512],
                                            start=True, stop=True,
                                            skip_group_check=True)
                                    nc.scalar.activation(
                                        pt[:, :, c0:512], st[:, :, c0:512],
                                        AF.Exp, scale=float(1.0 / np.sqrt(HALF)),
                                        bias=expb[:, 0:1])
                                    if qb * 512 <= k0:
                                        nc.vector.tensor_mul(
                                            pt[:, :, c0:c0 + P],
                                            pt[:, :, c0:c0 + P], dmb)
                                    pend.append((kc, pt))
                                    if len(pend) > 5:
                                        pv_mm(*pend.pop(0))
                                    if fill and kc % 2 == 1:
                                        fill.pop(0)()
                                for it in pend:
                                    pv_mm(*it)
                                _mark(nc, f"norm{h}.{qb}")
                                # normalize by softmax denominator (row
                                # 64); s=1 first: its shift-DMA is the tail
                                for s in (1, 0):
                                    dr = wk.tile([1, 512], f32, tag="dr",
                                                 bufs=4)
                                    nc.vector.reciprocal(
                                        dr[:], yps[s][HALF:HALF + 1, :])
                                    rb = wk.tile([HALF, 512], f32, tag="rb",
                                                 bufs=4)
                                    nc.gpsimd.partition_broadcast(
                                        rb[:], dr[:], channels=HALF)
                                    if s == 0:
                                        nc.vector.tensor_mul(
                                            yn[0:HALF, h, qb, :],
                                            yps[s][0:HALF, :], rb[:])
                                    else:
                                        ystg = wk.tile([HALF, 512], f16,
                                                       tag="ystg", bufs=2)
                                        nc.vector.tensor_mul(
                                            ystg[:], yps[s][0:HALF, :], rb[:])
                                        nc.sync.dma_start(
                                            yn[HALF:P, h, qb, :], ystg[:])
                            # final-rms stats for this head (lambda-weighted),
                            # on DVE: sqy = (yn * w) * yn, per q-block so the
                            # last chain after head 7 is short
                            for sq_qb in range(QB):
                                sqy = wk.tile([P, 512], f32, tag="sqy", bufs=2)
                                ynh = yn[:, h, sq_qb, :]
                                nc.vector.scalar_tensor_tensor(
                                    sqy[:], ynh, wsq[:, h:h + 1], ynh,
                                    mybir.AluOpType.mult, mybir.AluOpType.mult)
                                nc.gpsimd.tensor_add(
                                    ssqy_acc[:, sq_qb * 512:(sq_qb + 1) * 512],
                                    ssqy_acc[:, sq_qb * 512:(sq_qb + 1) * 512],
                                    sqy[:])
                            # stream one projection-weight chunk per head
                            nc.sync.dma_start(wp_s[:, h],
                                              wp_d[h * P:(h + 1) * P, :])

                        for h in range(HL):
                            fill = qkv_fillers(h + 1) if h < HL - 1 else ()
                            attention(h, fill)

                    # ---- final rms + pairwise collectives + projection ----
                    _mark(nc, "rmsy")
                    ssqb = wk.tile([P, S], f32, tag="ssqbc", bufs=1)
                    nc.gpsimd.partition_all_reduce(ssqb[:], ssqy_acc[:], 128,
                                                   bass_isa.ReduceOp.add)
                    nc.sync.dma_start(ssqy_in[:], ssqb[0:1, :])
                    if no_coll:
                        nc.sync.dma_start(ssqy_out[:], ssqb[0:1, :])
                    else:
                        nc.gpsimd.collective_compute(
                            "AllReduce", mybir.AluOpType.add,
                            ins=[ssqy_in.opt()], outs=[ssqy_out.opt()],
                            replica_groups=groups)
                    rry = wk.tile([P, TT], f32, tag="rry", bufs=1)
                    nc.sync.dma_start(
                        rry[:],
                        ssqy_out[0:1, :].rearrange("a (t p) -> (a p) t", p=P))
                    nc.scalar.activation(rry[:], rry[:], AF.Ln,
                                         scale=1.0 / DIM, bias=epsc[:, 0:1])
                    nc.scalar.activation(rry[:], rry[:], AF.Exp, scale=-0.5)

                    # projection reuses the big PSUM ring (no pool barrier
                    # between attention tail and first proj matmul)
                    _mark(nc, "proj")
                    for oc in range(4):
                        for t_ in range(TT):
                            psb = pbig.tile([P, QB, 512], f32, tag="big",
                                            bufs=3)
                            pso = psb[:, 0, :]
                            for c in range(HL):
                                nc.tensor.matmul(
                                    pso,
                                    yn[:, c, t_ // 4,
                                       (t_ % 4) * P:(t_ % 4 + 1) * P],
                                    wp_s[:, c, oc * 512:(oc + 1) * 512],
                                    start=(c == 0), stop=(c == HL - 1),
                                    skip_group_check=True)
                            # unscaled PSUM->SBUF copy first (DVE) so the
                            # PSUM ring never waits on the rry chain
                            osr = wk.tile([P, 512], f32, tag="osr", bufs=9)
                            nc.vector.tensor_copy(osr[:], pso)
                            osb = wk.tile([P, 512], f16, tag="osb", bufs=4)
                            nc.scalar.activation(osb[:], osr[:], AF.Copy,
                                                 scale=rry[:, t_:t_ + 1])
                            nc.sync.dma_start(
                                part_d[oc // 2, t_ * P:(t_ + 1) * P,
                                       (oc % 2) * 512:(oc % 2) * 512 + 512],
                                osb[:])
                            if no_coll and oc < 2:
                                # stub for the pairwise ReduceScatter:
                                # stream own-half partials to the output
                                nc.sync.dma_start(
                                    out_d[t_ * P:(t_ + 1) * P,
                                          oc * 512:(oc + 1) * 512],
                                    osb[:])
                    if not no_coll:
                        nc.gpsimd.collective_compute(
                            "ReduceScatter", mybir.AluOpType.add,
                            ins=[part_d.opt()], outs=[red_d.opt()],
                            replica_groups=groups)
                        nc.sync.dma_start(out_d[:, :], red_d[:])

            if dbg:
                nc.sync.dma_start(
                    dbg_qk.rearrange("p f (a b) -> p f a b", a=QB), qk16[:])
                nc.sync.dma_start(dbg_vp[:, :, :, :, :], vplus[:])
                nc.sync.dma_start(
                    dbg_yn.rearrange("p h (a b) -> p h a b", a=QB), yn[:])
                nc.sync.dma_start(dbg_ss[:], ssqy_in[:])
                nc.sync.dma_start(dbg_pp[:, :, :], part_d[:])

            free_x16()
            free_wp()
            free_wv()
            free_ssqy_acc()
            free_vplus()
            free_qk16()
            free_yn()

    nc.compile()
    _CACHE[key] = nc
    return nc


# ---------------- host wrapper ----------------

def _prep_inputs(x, w_qkv, w_proj, q_gain, diff_lambda):
    x = np.asarray(x, dtype=np.float32)
    wq = _ternary_quant(np.asarray(w_qkv, dtype=np.float32))
    wp = _ternary_quant(np.asarray(w_proj, dtype=np.float32))
    gain = np.asarray(q_gain, dtype=np.float32)
    lam = np.asarray(diff_lambda, dtype=np.float32)
    cpk, spk = _rope_tables()

    # causal mask for diagonal 128x128 blocks in scores^T layout:
    # element (key p, query j) valid iff j >= p
    dmask = (np.arange(P)[None, :] >= np.arange(P)[:, None]).astype(np.float16)
    dmask = np.ascontiguousarray(dmask)

    in_maps = []
    for core in range(N_CORES):
        b, hh = core // 2, core % 2
        q_rows = wq[hh * HL * HD:(hh + 1) * HL * HD].copy()    # [1024, 2048]
        k_rows = wq[QS + hh * KVL * HD: QS + (hh + 1) * KVL * HD]
        v_rows = wq[QS + KVS + hh * KVL * HD: QS + KVS + (hh + 1) * KVL * HD]
        gains = gain[hh * HL:(hh + 1) * HL]
        lams = lam[hh * HL:(hh + 1) * HL]

        # fold sign(gain) into the q weight rows, |gain| into the rms scale
        sg = np.sign(gains).astype(np.float32)
        sg[sg == 0] = 1.0
        q_rows *= np.repeat(sg, HD)[:, None]
        ag = np.maximum(np.abs(gains), 1e-30).astype(np.float32)

        wqk_T = np.ascontiguousarray(
            np.concatenate([q_rows, k_rows], axis=0).T.astype(np.float16))
        wv_T = np.ascontiguousarray(v_rows.T.astype(np.float16))
        x16 = np.ascontiguousarray(x[b].T.astype(np.float16))

        # projection weights with the differential combine + lambda folded:
        # y-dim order matches yn: per local head, [half1(64) | half2(64)]
        wpt = np.empty((HL * HD, DIM), np.float32)
        for i in range(HL):
            hg = hh * HL + i
            A = wp[:, hg * HD: hg * HD + HALF]           # [2048, 64]
            Bc = wp[:, hg * HD + HALF: (hg + 1) * HD]
            wpt[i * HD: i * HD + HALF] = (A + Bc).T
            wpt[i * HD + HALF: (i + 1) * HD] = (lams[i] * (Bc - A)).T
        wpt16 = np.ascontiguousarray(wpt.astype(np.float16))

        invg2 = np.full((P, FTOT), 1.0 / HD, np.float32)
        invg2[:, :HL] = (1.0 / (HD * ag * ag))[None, :]
        epsg = np.full((P, FTOT), EPS, np.float32)
        epsg[:, :HL] = (EPS / (ag * ag))[None, :]
        # stats weights (applied as (yn*w)*yn): 2 for half1, 2*lam^2 half2
        wsq = np.empty((P, HL), np.float32)
        wsq[0:HALF, :] = 2.0
        wsq[HALF:P, :] = (2.0 * lams * lams)[None, :]

        m = {
            "xT16": x16, "wqkT16": wqk_T, "wvT16": wv_T, "wpT16": wpt16,
            "cpk16": cpk, "spk16": spk,
            "invg2": np.ascontiguousarray(invg2),
            "epsg2": np.ascontiguousarray(epsg),
            "wsq": np.ascontiguousarray(wsq),
            "dmask16": dmask,
        }
        in_maps.append(m)
    return in_maps


def kernel(x, w_qkv, w_proj, q_gain, diff_lambda):
    nc = _build_program()
    in_maps = _prep_inputs(x, w_qkv, w_proj, q_gain, diff_lambda)
    last_err = None
    for attempt in range(3):
        try:
            res = bass_utils.run_bass_kernel_spmd(
                nc, in_maps, core_ids=list(range(N_CORES)))
            break
        except Exception as e:  # transient device wedges recover on retry
            last_err = e
            import time as _time
            _time.sleep(2.0)
    else:
        raise last_err
    out = np.empty((B, S, DIM), dtype=np.float32)
    for core in range(N_CORES):
        b, hh = core // 2, core % 2
        out[b, :, hh * OCOLS:(hh + 1) * OCOLS] = (
            res.results[core]["out"].astype(np.float32))
    return out
